# revision 26
# baseline (speedup 1.0000x reference)
"""CMHSA kernel for 8x TRN2 NeuronCores.

Sharding: data-parallel over the batch dim (B=8 -> one batch per core), no
collectives. Each core runs the full attention pipeline for its batch:

  xf = x[b] as [C, T]
  q = (scale*Wq) @ xf, k = Wk @ xf           (lhsT = host-pretransposed W)
  vT = xf^T @ Wv^T  -> [T, C]                (lhsT = xf, moving = Wv^T)
  per output head g:
    ST[t, q] = sum_{h,d} k[(h,d), t] * (head_w[g,h] * q[(h,d), q])
               (head conv fused into the score matmul via a K=384 stacked
               contraction; per-partition scaling of q on Pool)
    E = exp(ST) (ACT, straight from PSUM), Esq = E*E (Pool)
    AV matmul lhsT = [vT_g | ones]: rows 0..63 = E^T@v_g, row 64 = softmax
    denominators; ones-matmul on Esq -> per-q sum of E^2
  instance-norm folded into an affine on the AV output, computed PER HEAD
  so the stats/affine chain for head g hides under head g+1's matmuls:
    mean == 1/T exactly (softmax rows sum to 1)
    var from sum(E^2)/denom^2; rsqrt via reciprocal+sqrt + one Newton step
    out_g^T = (OT0 * (1/denom)) * rN + cN * colsum(v_g)
  colsum(v) comes from rowsum(x) pushed through the V projection.
  projection consumes the torch-style .view(B,T,C) reshape via stride-6
  access patterns over UT = concat_g out_g^T  (no data movement). It is
  emitted in 256-column chunks as soon as the heads each chunk reads are
  affined, so only the last chunk sits on the kernel tail.
  yT[c_out, t] = sum_cb projW_shuf[:, cb] @ UT[:, cb::6] + projb

float32r notes (probed against walrus): matmul inputs must be produced as
f32r (DMA from f32r DRAM, or DVE/ACT ops with f32r out); f32r matmuls
require dst base partition 0 and an even moving free dim; moving free dim
>= 256 or the PE drops to 1/4 rate.
"""

import numpy as np
from contextlib import ExitStack

import concourse.bass as bass
import concourse.bacc as bacc
import concourse.tile as tile
from concourse import mybir
from concourse.bass_utils import run_bass_kernel_spmd

B, C, H, W = 8, 384, 32, 32
NH, HD = 6, 64
T = H * W              # 1024
P = 128                # partitions
NB = C // P            # 3 channel blocks
TBN = T // P           # 8 t-blocks
EPS = 1e-5
SCALE = HD ** -0.5

F32 = mybir.dt.float32
F32R = mybir.dt.float32r
AF = mybir.ActivationFunctionType
OP = mybir.AluOpType
AX = mybir.AxisListType

# matmul input dtype: float32r streams 1 row/cycle (vs 4 for float32)
MM_DT = F32R
# post-softmax tiles (exp outputs) in bf16: same 1 row/cycle on PE, but
# half the SBUF traffic and 2x DVE throughput where DVE touches them
ET_DT = mybir.dt.bfloat16
ST_BUFS = 2

# projection chunks: [c0, c1) of yt columns, ready once `head` is affined
# (columns t of the chunk read ut cols u = 6t+cb, cb<6 -> heads u//T only).
# The last chunk is kept small so the kernel tail after head 5 is short.
PROJ_CHUNKS = [(0, 256, 1), (256, 512, 2), (512, 768, 4), (768, 852, 4),
               (852, 1024, 5)]


def build_kernel(tc, debug=False, repeats=1, ablate=()):
    nc = tc.nc
    ctx = ExitStack()

    xf_d = nc.dram_tensor("xf", [C, T], MM_DT, kind="ExternalInput").ap()
    wqt_d = nc.dram_tensor("wqt", [C, C], MM_DT, kind="ExternalInput").ap()
    wkt_d = nc.dram_tensor("wkt", [C, C], MM_DT, kind="ExternalInput").ap()
    wvt_d = nc.dram_tensor("wvt", [C, C], MM_DT, kind="ExternalInput").ap()
    pwts_d = nc.dram_tensor("pwts", [HD, NH * C], ET_DT, kind="ExternalInput").ap()
    vto_d = nc.dram_tensor("vto", [P, NH + 1], ET_DT, kind="ExternalInput").ap()
    wvec_d = nc.dram_tensor("wvec", [P, NH * NB], F32, kind="ExternalInput").ap()
    gam_d = nc.dram_tensor("gam", [1, NH], F32, kind="ExternalInput").ap()
    bet_d = nc.dram_tensor("bet", [1, NH], F32, kind="ExternalInput").ap()
    pjb_d = nc.dram_tensor("pjb", [P, NB], F32, kind="ExternalInput").ap()
    yt_d = nc.dram_tensor("yt", [C, T], F32, kind="ExternalOutput").ap()
    if debug:
        dbg = {
            "d_q0": nc.dram_tensor("d_q0", [P, T], F32, kind="ExternalOutput").ap(),
            "d_k0": nc.dram_tensor("d_k0", [P, T], F32, kind="ExternalOutput").ap(),
            "d_vta0": nc.dram_tensor("d_vta0", [P, NH * (HD + 1)], F32, kind="ExternalOutput").ap(),
            "d_sd": nc.dram_tensor("d_sd", [1, 2 * T], F32, kind="ExternalOutput").ap(),
            "d_rd": nc.dram_tensor("d_rd", [1, T], F32, kind="ExternalOutput").ap(),
            "d_vsum": nc.dram_tensor("d_vsum", [HD, NH], F32, kind="ExternalOutput").ap(),
            "d_rncn": nc.dram_tensor("d_rncn", [1, 2], F32, kind="ExternalOutput").ap(),
            "d_ut": nc.dram_tensor("d_ut", [HD, NH * T], ET_DT, kind="ExternalOutput").ap(),
            "d_et50": nc.dram_tensor("d_et50", [P, T], ET_DT, kind="ExternalOutput").ap(),
        }

    cons = ctx.enter_context(tc.tile_pool(name="cons", bufs=1))
    sb = ctx.enter_context(tc.tile_pool(name="sb", bufs=1))
    work = ctx.enter_context(tc.tile_pool(name="work", bufs=1))
    pp = ctx.enter_context(tc.tile_pool(name="pp", bufs=1, space="PSUM"))

    # ---- constant / persistent tiles -------------------------------------
    wqt = [cons.tile([P, C], MM_DT, tag=f"wqt{i}", name=f"wqt{i}") for i in range(NB)]
    wkt = [cons.tile([P, C], MM_DT, tag=f"wkt{i}", name=f"wkt{i}") for i in range(NB)]
    wvt = [cons.tile([P, C], MM_DT, tag=f"wvt{i}", name=f"wvt{i}") for i in range(NB)]
    pwts = cons.tile([HD, NH * C], ET_DT, tag="pwts")
    onesr = cons.tile([P, 2], ET_DT, tag="onesr")
    wvec = cons.tile([P, NH * NB], F32, tag="wvec")
    gam = cons.tile([1, NH], F32, tag="gam")
    bet = cons.tile([1, NH], F32, tag="bet")
    pjb = cons.tile([P, NB], F32, tag="pjb")

    xf = [sb.tile([P, T], MM_DT, tag=f"xf{i}", name=f"xf{i}") for i in range(NB)]
    qsb = [sb.tile([P, T], F32, tag=f"q{i}", name=f"q{i}") for i in range(NB)]
    ksb = [sb.tile([P, T], MM_DT, tag=f"k{i}", name=f"k{i}") for i in range(NB)]
    # vta[tb]: per head g, cols [g*65, g*65+64) = vT slice, col g*65+64 = 1.0
    vta = [sb.tile([P, NH * (HD + 1)], ET_DT, tag=f"vta{i}", name=f"vta{i}")
           for i in range(TBN)]
    ut = sb.tile([HD, NH * T], ET_DT, tag="ut")
    vsum_sb = sb.tile([HD, NH], F32, tag="vsum_sb")

    # wqt[0] first: it feeds the PE warmup matmuls that ramp the p-state
    # while the remaining input DMAs are still in flight
    nc.sync.dma_start(wqt[0][:, :], wqt_d[0 * P:1 * P, :])
    for i in range(NB):
        nc.sync.dma_start(xf[i][:, :], xf_d[i * P:(i + 1) * P, :])
    for i in range(NB):
        if i:
            nc.sync.dma_start(wqt[i][:, :], wqt_d[i * P:(i + 1) * P, :])
        nc.sync.dma_start(wkt[i][:, :], wkt_d[i * P:(i + 1) * P, :])
        nc.sync.dma_start(wvt[i][:, :], wvt_d[i * P:(i + 1) * P, :])
    nc.sync.dma_start(pwts[:, :], pwts_d[:, :])
    nc.sync.dma_start(onesr[:, :], vto_d[:, 0:2])
    nc.sync.dma_start(wvec[:, :], wvec_d[:, :])
    nc.sync.dma_start(gam[:, :], gam_d[:, :])
    nc.sync.dma_start(bet[:, :], bet_d[:, :])
    nc.sync.dma_start(pjb[:, :], pjb_d[:, :])

    for _rep in range(repeats):
        _build_body(tc, nc, debug, _rep, locals(), ablate)
    ctx.close()


def _build_body(tc, nc, debug, _rep, env, ablate=()):
    (cons, sb, work, pp, wqt, wkt, wvt, pwts, onesr, wvec, gam, bet, pjb,
     xf, qsb, ksb, vta, ut, vsum_sb, yt_d, vto_d) = (
        env["cons"], env["sb"], env["work"], env["pp"], env["wqt"], env["wkt"],
        env["wvt"], env["pwts"], env["onesr"], env["wvec"], env["gam"],
        env["bet"], env["pjb"], env["xf"], env["qsb"], env["ksb"], env["vta"],
        env["ut"], env["vsum_sb"], env["yt_d"], env["vto_d"])
    dbg = env.get("dbg")

    # ---- PE warmup: ~3us of throwaway matmuls on the first-loaded weight
    # tile so the PE p-state is fully ramped when the real work arrives.
    # Results land in the (not yet live) "av" PSUM region and are never read.
    if _rep == 0 and "warm" not in ablate:
        wps = pp.tile([P, C], F32, tag="av", bufs=1, name="warm_ps")
        for _ in range(20):
            nc.tensor.matmul(wps[:, :], lhsT=wqt[0][:, 0:P], rhs=wqt[0][:, :],
                             start=True, stop=True, skip_group_check=True)

    # ---- stage 1: Q/K projections [C, T]; V^T projection [T, C] ----------
    for mb in range(NB):
        for dst, wt in ((qsb, wqt), (ksb, wkt)):
            ps = pp.tile([P, T], F32, tag="st", bufs=ST_BUFS)
            for qh in range(2):
                for kb in range(NB):
                    nc.tensor.matmul(
                        ps[:, qh * 512:(qh + 1) * 512],
                        lhsT=wt[kb][:, mb * P:(mb + 1) * P],
                        rhs=xf[kb][:, qh * 512:(qh + 1) * 512],
                        start=(kb == 0), stop=(kb == NB - 1),
                    )
            nc.vector.tensor_copy(dst[mb][:, :], ps[:, :])

    for tb in range(TBN):
        ps = pp.tile([P, C], F32, tag="st", bufs=ST_BUFS)
        for kb in range(NB):
            nc.tensor.matmul(
                ps[:, :],
                lhsT=xf[kb][:, tb * P:(tb + 1) * P],
                rhs=wvt[kb][:, :],
                start=(kb == 0), stop=(kb == NB - 1),
            )
        # scatter v columns into the [vT_g | 1] interleaved layout
        vdst = vta[tb].rearrange("p (g c) -> p g c", c=HD + 1)
        nc.vector.tensor_copy(vdst[:, :, 0:HD], ps[:, :])
        nc.sync.dma_start(vdst[:, :, HD], vto_d[:, 0:NH])

    if debug:
        nc.sync.dma_start(dbg["d_q0"][:, :], qsb[0][:, :])
        nc.sync.dma_start(dbg["d_k0"][:, :], ksb[0].bitcast(F32)[:, :])
        nc.sync.dma_start(dbg["d_vta0"][:, :], vta[0].bitcast(F32)[:, :])

    # colsum(v)[c] = sum_c' xsum[c'] * WvT[c', c], xsum = rowsum(x)
    vs_ps = pp.tile([1, C], F32, tag="st", bufs=ST_BUFS, name="vs_ps")
    for kb in range(NB):
        xs32 = work.tile([P, 1], F32, tag="xs32", name="xs32")
        nc.vector.reduce_sum(xs32[:, :], xf[kb].bitcast(F32)[:, :], axis=AX.X)
        xs = work.tile([P, 1], MM_DT, tag="xs", name="xs")
        nc.vector.tensor_copy(xs[:, :], xs32[:, :])
        nc.tensor.matmul(vs_ps[:, :], lhsT=xs[:, :], rhs=wvt[kb][:, :],
                         start=(kb == 0), stop=(kb == NB - 1))
    vsrow = work.tile([1, C], F32, tag="vsrow")
    nc.vector.tensor_copy(vsrow[:, :], vs_ps[:, :])
    # [1, 384] row -> [64, 6] (partition=d, free=g): SBUF->SBUF partition
    # scatter is illegal, so bounce through DRAM where APs are unrestricted
    vsd = nc.dram_tensor(f"vsd{_rep}", [1, C], F32, kind="Internal").ap()
    nc.sync.dma_start(vsd[:, :], vsrow[:, :])
    vsr = vsd.rearrange("p (g d) -> p g d", d=HD)
    nc.sync.dma_start(vsum_sb[:, :], vsr[0, :, :].transpose([1, 0]))

    # ---- stages 3-5: per-head stats + affine; chunked projection ---------
    skip_tail = "tail" in ablate
    utr = ut.rearrange("p (t s) -> p t s", s=NH)

    def stats_affine(g, av, psq):
        # per-head instance-norm stats; runs while head g+1 streams on PE.
        # All row-stats live on partition 0 (partition_broadcast source).
        if skip_tail:
            return
        # free the psq PSUM bank early (Pool, off the critical path) so the
        # next head's sumsq accumulation never waits on this head's stats
        sd = work.tile([1, T], F32, tag="sd", bufs=2, name="sd")
        # (GPSIMD cannot access PSUM on HW: this copy must be DVE/ACT)
        if "sq" not in ablate:
            nc.vector.tensor_copy(sd[:, :], psq[0:1, :])
        else:
            nc.vector.tensor_copy(sd[:, :], av[HD:HD + 1, :])
        # denominators to partition 0 via ACT Copy (shares the Exp table).
        # NOTE: custom-DVE ops (reciprocal_approx_*) ignore partition
        # offsets on HW, so the recip must read a partition-0 tile.
        den = work.tile([1, T], F32, tag="den", bufs=2, name="den")
        nc.scalar.activation(den[:, :], av[HD:HD + 1, :], AF.Copy)
        rd = work.tile([1, T], F32, tag="rd", bufs=2, name="rd")
        nc.vector.reciprocal_approx_fast(rd[:, :], den[:, :])
        # var = (sum_q sumsq_q * rd_q^2)/T^2 - 1/T^2 + EPS; rd^2 on ACT
        # (Square shares the Exp table: no table switch), then a single
        # fused multiply+reduce with the EPS term as the reduce seed
        rdsq = work.tile([1, T], F32, tag="rdsq", bufs=2, name="rdsq")
        nc.scalar.activation(rdsq[:, :], rd[:, :], AF.Square)
        t6a = work.tile([1, T], F32, tag="t6a", bufs=2, name="t6a")
        nc.vector.tensor_tensor(t6a[:, :], rdsq[:, :], sd[:, :], op=OP.mult)
        s2 = work.tile([1, 1], F32, tag="s2", bufs=2, name="s2")
        nc.vector.reduce_sum(s2[:, :], t6a[:, :], axis=AX.X)
        var_e = work.tile([1, 1], F32, tag="var_e", bufs=2, name="var_e")
        nc.vector.tensor_scalar(
            var_e[:, :], s2[:, :],
            scalar1=1.0 / (T * T), scalar2=(EPS - 1.0 / (T * T)),
            op0=OP.mult, op1=OP.add,
        )
        # table-free rsqrt: quake seed via float arithmetic on the exponent
        # bits, then two Newton steps r <- r*(1.5 - 0.5*v*r^2)
        r0 = work.tile([1, 1], F32, tag="r0", bufs=2, name="r0")
        nc.vector.tensor_copy(r0[:, :], var_e.bitcast(mybir.dt.int32)[:, :])
        nc.vector.tensor_scalar(r0[:, :], r0[:, :], scalar1=-0.5,
                                scalar2=float(0x5f3759df), op0=OP.mult,
                                op1=OP.add)
        r0i = work.tile([1, 1], mybir.dt.int32, tag="r0i", bufs=2, name="r0i")
        nc.vector.tensor_copy(r0i[:, :], r0[:, :])
        r0f = r0i.bitcast(F32)
        t1 = work.tile([1, 1], F32, tag="t1", bufs=2, name="t1")
        for _ in range(2):
            nc.vector.tensor_tensor(t1[:, :], r0f[:, :], r0f[:, :], op=OP.mult)
            nc.vector.tensor_tensor(t1[:, :], t1[:, :], var_e[:, :], op=OP.mult)
            nc.vector.tensor_scalar(t1[:, :], t1[:, :], scalar1=-0.5,
                                    scalar2=1.5, op0=OP.mult, op1=OP.add)
            nc.vector.tensor_tensor(r0f[:, :], r0f[:, :], t1[:, :], op=OP.mult)
        rn = work.tile([1, 2], F32, tag="rn", bufs=2, name="rn")
        nc.vector.tensor_tensor(rn[:, 0:1], r0f[:, :], gam[0:1, g:g + 1],
                                op=OP.mult)
        nc.vector.tensor_scalar(rn[:, 1:2], rn[:, 0:1],
                                scalar1=-1.0 / T, scalar2=None, op0=OP.mult)
        nc.vector.tensor_tensor(rn[:, 1:2], rn[:, 1:2],
                                bet[0:1, g:g + 1], op=OP.add)
        # ---- affine on UT for this head
        usl = ut[:, g * T:(g + 1) * T]
        rdbc = work.tile([HD, T], F32, tag="rdbc", bufs=2, name="rdbc")
        nc.gpsimd.partition_broadcast(rdbc[:, :], rd[:, :])
        # the rd scale runs on idle Pool, fully parallel to the DVE variance
        # chain (emitted before rnbc so the Pool FIFO can't stall it);
        # only the rn/cn affine sits on the DVE tail
        nc.gpsimd.tensor_tensor(usl, usl, rdbc[:, :], op=OP.mult)
        rnbc = work.tile([HD, 2], F32, tag="rnbc", bufs=2, name="rnbc")
        nc.gpsimd.partition_broadcast(rnbc[:, :], rn[:, :])
        avec = work.tile([HD, 1], F32, tag="avec", bufs=2, name="avec")
        nc.vector.tensor_tensor(avec[:, :], vsum_sb[:, g:g + 1],
                                rnbc[:, 1:2], op=OP.mult)
        nc.vector.tensor_scalar(usl, usl,
                                scalar1=rnbc[:, 0:1], scalar2=avec[:, :],
                                op0=OP.mult, op1=OP.add)
        if debug and g == NH - 1:
            nc.sync.dma_start(dbg["d_sd"][:, :], sd[:, :])
            nc.sync.dma_start(dbg["d_rd"][:, :], rd[:, :])
            nc.sync.dma_start(dbg["d_vsum"][:, :], vsum_sb[:, :])
            nc.sync.dma_start(dbg["d_rncn"][:, :], rn[:, :])
            nc.sync.dma_start(dbg["d_ut"][:, :], ut[:, :])

    def proj_chunk(c0, c1):
        # projection columns [c0, c1) of yT; rhs strides over UT by 6.
        # yps rides the "st" PSUM region between score tiles.
        w = c1 - c0
        for mb in range(NB):
            yps = pp.tile([P, T], F32, tag="st", bufs=ST_BUFS, name="yps")
            for cb in range(NH):
                nc.tensor.matmul(
                    yps[:, 0:w],
                    lhsT=pwts[:, cb * C + mb * P:cb * C + (mb + 1) * P],
                    rhs=utr[:, c0:c1, cb],
                    start=(cb == 0), stop=(cb == NH - 1),
                )
            ysb = work.tile([P, 512], F32, tag="ysb", bufs=2, name="ysb")
            nc.vector.tensor_scalar(ysb[:, 0:w], yps[:, 0:w],
                                    scalar1=pjb[:, mb:mb + 1], scalar2=None,
                                    op0=OP.add)
            nc.sync.dma_start(yt_d[mb * P:(mb + 1) * P, c0:c1], ysb[:, 0:w])

    # ---- stage 2: per output head: scores + softmax + AV -----------------
    def make_qq(g):
        if "qq" in ablate:
            return [q.bitcast(MM_DT) for q in qsb]
        qq = [work.tile([P, T], MM_DT, tag=f"qq{kb}", bufs=2, name=f"qq{kb}")
              for kb in range(NB)]
        for kb in range(NB):
            nc.gpsimd.tensor_scalar(
                qq[kb][:, :], qsb[kb][:, :],
                scalar1=wvec[:, g * NB + kb:g * NB + kb + 1], scalar2=None,
                op0=OP.mult,
            )
        return qq

    qq_next = make_qq(0)
    for g in range(NH):
        qq = qq_next
        av = pp.tile([HD + 1, T], F32, tag="av", bufs=1)
        psq = None if "sq" in ablate else pp.tile([1, T], F32, tag="sq", bufs=1)

        def consume(tb, et, esq):
            # AV + sumsq matmuls for a finished tile; emitted one tile late
            # so the PE FIFO never head-of-line blocks on ACT's exp outputs
            for qh in range(2):
                sl = slice(qh * 512, (qh + 1) * 512)
                nc.tensor.matmul(
                    av[0:HD + 1, sl],
                    lhsT=vta[tb][:, g * (HD + 1):(g + 1) * (HD + 1)],
                    rhs=et[:, sl],
                    start=(tb == 0), stop=(tb == TBN - 1),
                    skip_group_check=True,
                )
                if "sq" not in ablate:
                    nc.tensor.matmul(
                        psq[0:1, sl],
                        lhsT=onesr[:, 0:1],
                        rhs=esq[:, sl],
                        start=(tb == 0), stop=(tb == TBN - 1),
                        skip_group_check=True,
                    )

        pend = None
        for tb in range(TBN):
            st = pp.tile([P, T], F32, tag="st", bufs=ST_BUFS)
            for qh in range(2):
                for kb in range(NB):
                    nc.tensor.matmul(
                        st[:, qh * 512:(qh + 1) * 512],
                        lhsT=ksb[kb][:, tb * P:(tb + 1) * P],
                        rhs=qq[kb][:, qh * 512:(qh + 1) * 512],
                        start=(kb == 0), stop=(kb == NB - 1),
                    )
            et = work.tile([P, T], ET_DT, tag="et", bufs=3)
            esq = work.tile([P, T], ET_DT, tag="esq", bufs=3)
            nc.scalar.activation(et[:, :], st[:, :], AF.Exp)
            if "sq" not in ablate:
                if tb % 3 == 1:
                    # exp(2s) straight from PSUM: no dependency on et
                    nc.scalar.activation(esq[:, :], st[:, :], AF.Exp, scale=2.0)
                else:
                    # bf16 et*et on DVE runs in 2x mode
                    nc.vector.tensor_tensor(esq[:, :], et[:, :], et[:, :],
                                            op=OP.mult)
            if debug and g == 5 and tb == 0:
                nc.sync.dma_start(dbg["d_et50"][:, :], et[:, :])
            if pend is not None:
                consume(*pend)
            pend = (tb, et, esq)
            # projection chunks that became ready after the PREVIOUS head's
            # affine; emitted deep into this head's stream so the affine
            # chain has certainly drained and PE never stalls on it
            if tb == 5:
                for c0, c1, ready in PROJ_CHUNKS:
                    if ready == g - 1:
                        proj_chunk(c0, c1)
        consume(*pend)
        # prescale q for the next head BEFORE the stats chain so the Pool
        # queue never makes the next head's first score matmul wait
        if g + 1 < NH:
            qq_next = make_qq(g + 1)
        # raw AV rows into UT, then per-head stats + affine
        nc.vector.tensor_copy(ut[:, g * T:(g + 1) * T], av[0:HD, :])
        stats_affine(g, av, psq)
        if g == NH - 1:
            for c0, c1, ready in PROJ_CHUNKS:
                if ready == g:
                    proj_chunk(c0, c1)


_CACHED = {}


def _get_nc(debug=False):
    if debug not in _CACHED:
        nc = bacc.Bacc("TRN2", target_bir_lowering=False, debug=False,
                       num_devices=B)
        with tile.TileContext(nc) as tc:
            build_kernel(tc, debug=debug)
        nc.compile()
        _CACHED[debug] = nc
    return _CACHED[debug]


def prep_inputs(x, Wq, Wk, Wv, head_w, gamma, beta, projW, projb):
    import ml_dtypes
    x = np.ascontiguousarray(x, dtype=np.float32)
    xfs = x.reshape(B, C, T)
    wqt = np.ascontiguousarray((Wq * SCALE).T, dtype=np.float32)
    wkt = np.ascontiguousarray(Wk.T, dtype=np.float32)
    wvt = np.ascontiguousarray(Wv.T, dtype=np.float32)
    pwts = np.empty((HD, NH * C), dtype=ml_dtypes.bfloat16)
    for cb in range(NH):
        pwts[:, cb * C:(cb + 1) * C] = projW[:, cb * HD:(cb + 1) * HD].T
    vto = np.ones((P, NH + 1), dtype=ml_dtypes.bfloat16)
    wvec = np.empty((P, NH * NB), dtype=np.float32)
    for g in range(NH):
        for kb in range(NB):
            rows = (kb * P + np.arange(P)) // HD
            wvec[:, g * NB + kb] = head_w[g, rows]
    gam = np.ascontiguousarray(gamma.reshape(1, NH), dtype=np.float32)
    bet = np.ascontiguousarray(beta.reshape(1, NH), dtype=np.float32)
    pjb = np.ascontiguousarray(projb.reshape(NB, P).T, dtype=np.float32)
    shared = dict(wqt=wqt, wkt=wkt, wvt=wvt, pwts=pwts, vto=vto, wvec=wvec,
                  gam=gam, bet=bet, pjb=pjb)
    return [dict(xf=np.ascontiguousarray(xfs[i]), **shared) for i in range(B)]


def run(in_maps, debug=False, **kw):
    nc = _get_nc(debug=debug)
    return run_bass_kernel_spmd(nc, in_maps, core_ids=list(range(B)), **kw)


_RT = None


def _get_rt():
    """Build the SPMD executable ONCE and keep it: repeated kernel() calls
    skip retrace/recompile. xf is sharded over the 8 cores; all weight
    tensors ride replicated (transferred once, not 8x). The executable is
    warmed with a zero-input run so cold device state (activation / custom
    op tables) never degrades the first real call."""
    global _RT
    if _RT is not None:
        return _RT
    import jax
    from jax.experimental.shard_map import shard_map
    from jax.sharding import Mesh, PartitionSpec, NamedSharding
    from concourse import bass2jax

    nc = _get_nc()
    bass2jax.install_neuronx_cc_hook()
    partition_name = (nc.partition_id_tensor.name
                      if nc.partition_id_tensor else None)
    in_names, out_names, out_avals = [], [], []
    for alloc in nc.m.functions[0].allocations:
        if not isinstance(alloc, mybir.MemoryLocationSet):
            continue
        name = alloc.memorylocations[0].name
        if alloc.kind == "ExternalInput":
            if name != partition_name:
                in_names.append(name)
        elif alloc.kind == "ExternalOutput":
            out_names.append(name)
            out_avals.append(jax.core.ShapedArray(
                tuple(alloc.tensor_shape), mybir.dt.np(alloc.dtype)))
    all_in = list(in_names) + list(out_names)
    if partition_name is not None:
        all_in.append(partition_name)

    def _body(*args):
        operands = list(args)
        if partition_name is not None:
            operands.append(bass2jax.partition_id_tensor())
        return tuple(bass2jax._bass_exec_p.bind(
            *operands,
            out_avals=tuple(out_avals),
            in_names=tuple(all_in),
            out_names=tuple(out_names),
            lowering_input_output_aliases=(),
            sim_require_finite=True,
            sim_require_nnan=True,
            nc=nc,
        ))

    mesh = Mesh(np.asarray(jax.devices()[:B]), ("core",))
    in_specs = tuple(PartitionSpec("core") if n == "xf" else PartitionSpec()
                     for n in in_names)
    in_specs = in_specs + (PartitionSpec("core"),) * len(out_names)
    out_specs = (PartitionSpec("core"),) * len(out_names)
    fn = jax.jit(shard_map(_body, mesh=mesh, in_specs=in_specs,
                           out_specs=out_specs, check_rep=False))
    sh_core = NamedSharding(mesh, PartitionSpec("core"))
    zeros = [jax.device_put(
        np.zeros((B * av.shape[0], *av.shape[1:]), av.dtype), sh_core)
        for av in out_avals]
    jax.block_until_ready(zeros)
    # warm run: loads the NEFF + activation/custom-op tables on every core
    dt_of = {}
    for alloc in nc.m.functions[0].allocations:
        if isinstance(alloc, mybir.MemoryLocationSet) \
                and alloc.kind == "ExternalInput":
            dt_of[alloc.memorylocations[0].name] = mybir.dt.np(alloc.dtype)
    warm_in = []
    for n in in_names:
        shp, dt = _INPUT_SHAPES[n], dt_of[n]
        if n == "xf":
            shp = (B * shp[0], *shp[1:])
        warm_in.append(np.zeros(shp, dt))
    jax.block_until_ready(fn(*warm_in, *zeros))
    _RT = (fn, in_names, out_avals, zeros)
    return _RT


_INPUT_SHAPES = {
    "xf": (C, T), "wqt": (C, C), "wkt": (C, C), "wvt": (C, C),
    "pwts": (HD, NH * C), "vto": (P, NH + 1), "wvec": (P, NH * NB),
    "gam": (1, NH), "bet": (1, NH), "pjb": (P, NB),
}


def kernel(**inputs):
    in_maps = prep_inputs(**inputs)
    try:
        fn, in_names, out_avals, zeros = _get_rt()
    except Exception:
        res = run(in_maps)
        out = np.stack([res.results[i]["yt"].reshape(C, H, W)
                        for i in range(B)])
        return out.astype(np.float32)
    arrs = []
    for n in in_names:
        if n == "xf":
            arrs.append(np.concatenate(
                [in_maps[c]["xf"] for c in range(B)], axis=0))
        else:
            arrs.append(in_maps[0][n])
    outs = fn(*arrs, *zeros)
    yt = np.asarray(outs[0]).reshape(B, C, H, W)
    return yt.astype(np.float32)
